# revision 7
# baseline (speedup 1.0000x reference)
"""GIN-style 3-layer GNN encoder on 8 Trainium2 NeuronCores (Bass/Tile).

Reference computation (fp32):
    h = x @ W_in.T + b_in                                  [50000, 96]
    for l in 0..2:
        agg = segment_sum(h[src], dst, N)                  [50000, 96]
        h = (h + agg) @ W_layers[l].T + b_layers[l]
    out = concat([h0..h3], 1) @ W_out.T + b_out            [50000, 128]

Distribution: nodes are partitioned contiguously across the 8 cores
(6250/core); each edge is owned by the core that owns its dst node.  Each
layer, the updated node features are AllGathered into a replicated
row-major fp16 table h_full [50000, 128pad] in each core's HBM.

Per-core segment sum: a core's node range is split into 49 windows of 128
nodes.  Every window gets a fixed number of 128-edge tiles (T_lo tiles
whose src < 32768, T_hi tiles with src >= 32768 — the split keeps gather
indices inside int16 range; host pads with dummy edges src=0 / dst=-1).
Edge features are fetched with gpsimd dma_gather (fp16 256B rows, edge i
of a gather lands in partition i%128).  For each window the one-hot
matrix onehot[e, t, j] = (dst_local[e, t] == j) is built on DVE with a
single broadcast is_equal per window, and the PE computes
    psum[96, 128] += gathered_tile[128e, 96].T @ onehot_tile[128e, 128]
which is exactly aggT for the window.  h+agg, the layer matmul (fp32r),
bias (ACT) follow; a PE transpose produces the row-major fp16 shard for
the next AllGather.
"""
import sys

sys.path.insert(0, "/opt/trn_rl_repo")

import numpy as np

N_NODES = 50000
N_EDGES = 800000
IN_DIM = 128
HID = 96
OUT_DIM = 128
N_LAYERS = 3
N_CORES = 8
NPC = N_NODES // N_CORES          # 6250 nodes per core
WIN = 128                         # window width (nodes)
NW = (NPC + WIN - 1) // WIN       # 49 windows per core (last = 106 nodes)
SPLIT = 32768                     # int16 index split for gather
CHUNK_W = 5                       # windows per gather chunk
CW_N = 512                        # node-chunk for dense matmuls

_cache = {}


def _prep(edge_index):
    """Host-side edge bucketing -> per-core gather index / dst tables."""
    src = edge_index[0].astype(np.int64)
    dst = edge_index[1].astype(np.int64)
    core = dst // NPC
    din = dst % NPC
    w = din // WIN
    dstl = din % WIN
    is_hi = (src >= SPLIT).astype(np.int64)

    key = (core * NW + w) * 2 + is_hi
    order = np.argsort(key, kind="stable")
    s_src = src[order]
    s_dstl = dstl[order]
    s_key = key[order]
    s_hi = is_hi[order]

    counts = np.bincount(key, minlength=N_CORES * NW * 2)
    T_lo = max(1, int(-(-counts.reshape(-1, 2)[:, 0].max() // 128)))
    T_hi = max(1, int(-(-counts.reshape(-1, 2)[:, 1].max() // 128)))
    T = T_lo + T_hi

    starts = np.zeros(N_CORES * NW * 2, np.int64)
    starts[1:] = np.cumsum(counts)[:-1]
    pos = np.arange(len(s_key)) - starts[s_key]

    c_arr = s_key // (2 * NW)
    w_arr = (s_key // 2) % NW

    idx_lo = np.zeros((N_CORES, NW, T_lo * 128), np.int16)
    idx_hi = np.zeros((N_CORES, NW, T_hi * 128), np.int16)
    dstl_arr = np.full((N_CORES, NW, T, 128), -1.0, np.float16)

    lo_m = s_hi == 0
    flat = (c_arr[lo_m] * NW + w_arr[lo_m]) * (T_lo * 128) + pos[lo_m]
    idx_lo.reshape(-1)[flat] = s_src[lo_m].astype(np.int16)
    t_g = pos[lo_m] // 128
    e_g = pos[lo_m] % 128
    flat = ((c_arr[lo_m] * NW + w_arr[lo_m]) * T + t_g) * 128 + e_g
    dstl_arr.reshape(-1)[flat] = s_dstl[lo_m].astype(np.float16)

    hi_m = ~lo_m
    flat = (c_arr[hi_m] * NW + w_arr[hi_m]) * (T_hi * 128) + pos[hi_m]
    idx_hi.reshape(-1)[flat] = (s_src[hi_m] - SPLIT).astype(np.int16)
    t_g = pos[hi_m] // 128 + T_lo
    e_g = pos[hi_m] % 128
    flat = ((c_arr[hi_m] * NW + w_arr[hi_m]) * T + t_g) * 128 + e_g
    dstl_arr.reshape(-1)[flat] = s_dstl[hi_m].astype(np.float16)

    def wrap(vals):  # [NW*Tc*128] -> [128, NW*Tc*8] int16 wrapped+replicated
        v = vals.reshape(-1, 16).T  # [16, n/16]
        return np.tile(v, (8, 1)).copy()

    idx_lo_w = np.stack([wrap(idx_lo[c].reshape(-1)) for c in range(N_CORES)])
    idx_hi_w = np.stack([wrap(idx_hi[c].reshape(-1)) for c in range(N_CORES)])
    dstloc = np.ascontiguousarray(dstl_arr.transpose(0, 3, 1, 2))  # [C,128,NW,T]
    return idx_lo_w, idx_hi_w, dstloc, T_lo, T_hi


def _build(T_lo, T_hi):
    from concourse import bass, bacc, tile, mybir, library_config

    dt = mybir.dt
    T = T_lo + T_hi
    nc = bacc.Bacc("TRN2", target_bir_lowering=False, debug=False,
                   num_devices=N_CORES)

    # ---- I/O ----
    xT_in = nc.dram_tensor("xT", [IN_DIM, NPC], dt.float32, kind="ExternalInput")
    w_inT_in = nc.dram_tensor("w_inT", [IN_DIM, HID], dt.float32,
                              kind="ExternalInput")
    b_in_in = nc.dram_tensor("b_in", [HID, 1], dt.float32, kind="ExternalInput")
    w_lT_in = nc.dram_tensor("w_lT", [N_LAYERS, HID, HID], dt.float32,
                             kind="ExternalInput")
    b_l_in = nc.dram_tensor("b_l", [N_LAYERS, HID, 1], dt.float32,
                            kind="ExternalInput")
    w_out4_in = nc.dram_tensor("w_out4", [N_LAYERS + 1, HID, OUT_DIM],
                               dt.float16, kind="ExternalInput")
    b_out_in = nc.dram_tensor("b_out", [OUT_DIM, 1], dt.float32,
                              kind="ExternalInput")
    iota_in = nc.dram_tensor("iota", [128, WIN], dt.float16,
                             kind="ExternalInput")
    id96_in = nc.dram_tensor("id96", [HID, HID], dt.float16,
                             kind="ExternalInput")
    id128_in = nc.dram_tensor("id128", [128, 128], dt.float32,
                              kind="ExternalInput")
    idx_lo_in = nc.dram_tensor("idx_lo", [128, NW * T_lo * 8], dt.int16,
                               kind="ExternalInput")
    idx_hi_in = nc.dram_tensor("idx_hi", [128, NW * T_hi * 8], dt.int16,
                               kind="ExternalInput")
    dstloc_in = nc.dram_tensor("dstloc", [128, NW, T], dt.float16,
                               kind="ExternalInput")
    out_ext = nc.dram_tensor("out", [NPC, OUT_DIM], dt.float32,
                             kind="ExternalOutput")

    f32, f32r, f16 = dt.float32, dt.float32r, dt.float16

    with tile.TileContext(nc, num_cores=N_CORES) as tc:
        nc.gpsimd.load_library(library_config.mlp)
        with tc.tile_pool(name="persist", bufs=1) as pp, \
             tc.tile_pool(name="xpool", bufs=3) as xpool, \
             tc.tile_pool(name="glo", bufs=2) as glo_pool, \
             tc.tile_pool(name="ghi", bufs=2) as ghi_pool, \
             tc.tile_pool(name="oh", bufs=3) as oh_pool, \
             tc.tile_pool(name="otile", bufs=2) as ot_pool, \
             tc.tile_pool(name="ps_agg", bufs=3, space="PSUM") as ps_agg, \
             tc.tile_pool(name="ps_big", bufs=2, space="PSUM") as ps_big, \
             tc.tile_pool(name="ps_tr", bufs=2, space="PSUM") as ps_tr, \
             tc.tile_pool(name="dram", bufs=1, space="DRAM") as dram:

            # ---- persistent SBUF ----
            def load(name, shape, dtype, src_ap):
                t = pp.tile(shape, dtype, name=name)
                nc.sync.dma_start(out=t[:], in_=src_ap)
                return t

            w_inT = load("w_inT", [IN_DIM, HID], f32r, w_inT_in[:].bitcast(f32r))
            b_in = load("b_in", [HID, 1], f32, b_in_in[:])
            w_lT = [load(f"w_lT{l}", [HID, HID], f32r, w_lT_in[l].bitcast(f32r))
                    for l in range(N_LAYERS)]
            b_l = [load(f"b_l{l}", [HID, 1], f32, b_l_in[l])
                   for l in range(N_LAYERS)]
            w_out4 = [load(f"w_out4_{s}", [HID, OUT_DIM], f16, w_out4_in[s])
                      for s in range(N_LAYERS + 1)]
            b_out = load("b_out", [OUT_DIM, 1], f32, b_out_in[:])
            iota = load("iota", [128, WIN], f16, iota_in[:])
            id96 = load("id96", [HID, HID], f16, id96_in[:])
            id128 = load("id128", [128, 128], f32, id128_in[:])
            idx_lo = load("idx_lo", [128, NW * T_lo * 8], dt.int16, idx_lo_in[:])
            idx_hi = load("idx_hi", [128, NW * T_hi * 8], dt.int16, idx_hi_in[:])
            dstloc = load("dstloc", [128, NW, T], f16, dstloc_in[:])

            h_state = [pp.tile([HID, NPC], f16, name=f"h{s}")
                       for s in range(N_LAYERS + 1)]
            h_plus = pp.tile([HID, NPC], f32r, name="h_plus")
            rm_buf = pp.tile([128, NW, 128], f16, name="rm_buf")

            h_full = dram.tile([N_NODES, 128], f16)
            bounce = dram.tile([NPC, 128], f16)

            node_chunks = [(j * CW_N, min(CW_N, NPC - j * CW_N))
                           for j in range(-(-NPC // CW_N))]

            def epilogue(s):
                """h_state[s] -> row-major fp16 shard -> AllGather h_full."""
                for t in range(NW):
                    n0 = t * 128
                    tn = min(128, NPC - n0)
                    pst = ps_tr.tile([128, HID], f16, name="pst")
                    nc.tensor.transpose(pst[:tn, :],
                                        h_state[s][:, n0:n0 + tn], id96[:])
                    nc.scalar.copy(rm_buf[:tn, t, 0:HID], pst[:tn, :])
                nc.sync.dma_start(
                    out=bounce[0:(NW - 1) * 128, :].rearrange(
                        "(t p) d -> p t d", p=128),
                    in_=rm_buf[:, 0:NW - 1, :])
                tn = NPC - (NW - 1) * 128
                nc.sync.dma_start(out=bounce[(NW - 1) * 128:NPC, :],
                                  in_=rm_buf[:tn, NW - 1, :])
                nc.gpsimd.collective_compute(
                    "AllGather", mybir.AluOpType.bypass,
                    ins=[bounce.opt()], outs=[h_full.opt()],
                    replica_groups=[list(range(N_CORES))])

            # ---- input projection ----
            for n0, cw in node_chunks:
                xb = xpool.tile([IN_DIM, CW_N], f32r, name="xb")
                nc.sync.dma_start(out=xb[:, :cw],
                                  in_=xT_in[:, n0:n0 + cw].bitcast(f32r))
                ps = ps_big.tile([HID, CW_N], f32, name="psb")
                nc.tensor.matmul(ps[:, :cw], w_inT[:], xb[:, :cw],
                                 start=True, stop=True)
                nc.scalar.add(h_state[0][:, n0:n0 + cw], ps[:, :cw], b_in[:])
            epilogue(0)

            # ---- GIN layers ----
            h_lo = h_full[0:SPLIT, :]
            h_hi = h_full[SPLIT:N_NODES, :]
            w_chunks = [(c0, min(CHUNK_W, NW - c0))
                        for c0 in range(0, NW, CHUNK_W)]
            for l in range(N_LAYERS):
                for c0, cw in w_chunks:
                    g_lo = glo_pool.tile([128, CHUNK_W * T_lo, 128], f16,
                                         name="g_lo")
                    nc.gpsimd.dma_gather(
                        g_lo[:, :cw * T_lo, :], h_lo,
                        idx_lo[:, c0 * T_lo * 8:(c0 + cw) * T_lo * 8],
                        num_idxs=cw * T_lo * 128,
                        num_idxs_reg=cw * T_lo * 128, elem_size=128,
                        single_packet=False)
                    g_hi = ghi_pool.tile([128, CHUNK_W * T_hi, 128], f16,
                                         name="g_hi")
                    nc.gpsimd.dma_gather(
                        g_hi[:, :cw * T_hi, :], h_hi,
                        idx_hi[:, c0 * T_hi * 8:(c0 + cw) * T_hi * 8],
                        num_idxs=cw * T_hi * 128,
                        num_idxs_reg=cw * T_hi * 128, elem_size=128,
                        single_packet=False)
                    for wl in range(cw):
                        w_i = c0 + wl
                        n0 = w_i * 128
                        wn = min(128, NPC - n0)
                        oh = oh_pool.tile([128, T, WIN], f16, name="oh")
                        nc.vector.tensor_tensor(
                            oh[:],
                            iota[:].unsqueeze(1).broadcast_to([128, T, WIN]),
                            dstloc[:, w_i, :].unsqueeze(2)
                                .broadcast_to([128, T, WIN]),
                            mybir.AluOpType.is_equal)
                        ps = ps_agg.tile([HID, WIN], f32, name="psa")
                        for t in range(T):
                            if t < T_lo:
                                lhsT = g_lo[:, wl * T_lo + t, 0:HID]
                            else:
                                lhsT = g_hi[:, wl * T_hi + (t - T_lo), 0:HID]
                            nc.tensor.matmul(ps[:], lhsT, oh[:, t, :],
                                             start=(t == 0),
                                             stop=(t == T - 1))
                        nc.vector.tensor_tensor(
                            h_plus[:, n0:n0 + wn], ps[:, :wn],
                            h_state[l][:, n0:n0 + wn], mybir.AluOpType.add)
                for n0, cw in node_chunks:
                    ps = ps_big.tile([HID, CW_N], f32, name="psb")
                    nc.tensor.matmul(ps[:, :cw], w_lT[l][:],
                                     h_plus[:, n0:n0 + cw],
                                     start=True, stop=True)
                    nc.scalar.add(h_state[l + 1][:, n0:n0 + cw], ps[:, :cw],
                                  b_l[l][:])
                if l < N_LAYERS - 1:
                    epilogue(l + 1)

            # ---- output projection ----
            for n0, cw in node_chunks:
                ps = ps_big.tile([OUT_DIM, CW_N], f32, name="pso", tag="psb")
                for s in range(N_LAYERS + 1):
                    nc.tensor.matmul(ps[:, :cw], w_out4[s][:],
                                     h_state[s][:, n0:n0 + cw],
                                     start=(s == 0), stop=(s == N_LAYERS))
                ot = ot_pool.tile([OUT_DIM, CW_N], f32, name="ot")
                nc.scalar.add(ot[:, :cw], ps[:, :cw], b_out[:])
                for tt in range(-(-cw // 128)):
                    t0 = tt * 128
                    tn = min(128, cw - t0)
                    pst = ps_tr.tile([128, 128], f32, name="psto", tag="pst")
                    nc.tensor.transpose(pst[:tn, :], ot[:, t0:t0 + tn],
                                        id128[:])
                    orow = ot_pool.tile([128, 128], f32, name="orow")
                    nc.scalar.copy(orow[:tn, :], pst[:tn, :])
                    nc.sync.dma_start(
                        out=out_ext[n0 + t0:n0 + t0 + tn, :],
                        in_=orow[:tn, :])

    nc.compile()
    return nc


def _get_nc_and_inputs(inputs):
    from concourse import bass_utils  # noqa: F401  (path setup)

    x = np.asarray(inputs["x"], np.float32)
    edge_index = np.asarray(inputs["edge_index"], np.int32)
    W_in = np.asarray(inputs["W_in"], np.float32)
    b_in = np.asarray(inputs["b_in"], np.float32)
    W_layers = np.asarray(inputs["W_layers"], np.float32)
    b_layers = np.asarray(inputs["b_layers"], np.float32)
    W_out = np.asarray(inputs["W_out"], np.float32)
    b_out = np.asarray(inputs["b_out"], np.float32)

    idx_lo_w, idx_hi_w, dstloc, T_lo, T_hi = _prep(edge_index)

    key = ("nc", T_lo, T_hi)
    if key not in _cache:
        _cache.clear()
        _cache[key] = _build(T_lo, T_hi)
    nc = _cache[key]

    xT = np.ascontiguousarray(x.T)
    w_inT = np.ascontiguousarray(W_in.T)
    w_lT = np.ascontiguousarray(W_layers.transpose(0, 2, 1))
    b_l = np.ascontiguousarray(b_layers[:, :, None])
    w_out4 = np.ascontiguousarray(
        np.stack([W_out[:, s * HID:(s + 1) * HID].T
                  for s in range(N_LAYERS + 1)])).astype(np.float16)
    iota = np.tile(np.arange(WIN, dtype=np.float16), (128, 1))
    id96 = np.eye(HID, dtype=np.float16)
    id128 = np.eye(128, dtype=np.float32)

    in_maps = []
    for c in range(N_CORES):
        in_maps.append({
            "xT": np.ascontiguousarray(xT[:, c * NPC:(c + 1) * NPC]),
            "w_inT": w_inT,
            "b_in": b_in.reshape(HID, 1),
            "w_lT": w_lT,
            "b_l": b_l,
            "w_out4": w_out4,
            "b_out": b_out.reshape(OUT_DIM, 1),
            "iota": iota,
            "id96": id96,
            "id128": id128,
            "idx_lo": idx_lo_w[c],
            "idx_hi": idx_hi_w[c],
            "dstloc": dstloc[c],
        })
    return nc, in_maps


def run(inputs, trace=False):
    from concourse import bass_utils

    nc, in_maps = _get_nc_and_inputs(inputs)
    res = bass_utils.run_bass_kernel_spmd(
        nc, in_maps, core_ids=list(range(N_CORES)), trace=trace)
    out = np.concatenate([res.results[c]["out"] for c in range(N_CORES)], 0)
    return out, res


def kernel(**inputs):
    out, _ = run(inputs, trace=False)
    return out


# revision 8
# speedup vs baseline: 1.0082x; 1.0082x over previous
"""GIN-style 3-layer GNN encoder on 8 Trainium2 NeuronCores (Bass/Tile).

Reference computation (fp32):
    h = x @ W_in.T + b_in                                  [50000, 96]
    for l in 0..2:
        agg = segment_sum(h[src], dst, N)                  [50000, 96]
        h = (h + agg) @ W_layers[l].T + b_layers[l]
    out = concat([h0..h3], 1) @ W_out.T + b_out            [50000, 128]

Distribution: nodes are partitioned contiguously across the 8 cores
(6250/core); each edge is owned by the core that owns its dst node.  Each
layer, the updated node features are AllGathered into a replicated
row-major fp16 table h_full [50000, 128pad] in each core's HBM.

Per-core segment sum: a core's node range is split into 49 windows of 128
nodes.  Every window gets a fixed number of 128-edge tiles (T_lo tiles
whose src < 32768, T_hi tiles with src >= 32768 — the split keeps gather
indices inside int16 range; host pads with dummy edges src=0 / dst=-1).
Edge features are fetched with gpsimd dma_gather (fp16 256B rows, edge i
of a gather lands in partition i%128).  For each window the one-hot
matrix onehot[e, t, j] = (dst_local[e, t] == j) is built on DVE with a
single broadcast is_equal per window, and the PE computes
    psum[96, 128] += gathered_tile[128e, 96].T @ onehot_tile[128e, 128]
which is exactly aggT for the window.  h+agg, the layer matmul (fp32r),
bias (ACT) follow; a PE transpose produces the row-major fp16 shard for
the next AllGather.
"""
import sys

sys.path.insert(0, "/opt/trn_rl_repo")

import numpy as np

N_NODES = 50000
N_EDGES = 800000
IN_DIM = 128
HID = 96
OUT_DIM = 128
N_LAYERS = 3
N_CORES = 8
NPC = N_NODES // N_CORES          # 6250 nodes per core
WIN = 128                         # window width (nodes)
NW = (NPC + WIN - 1) // WIN       # 49 windows per core (last = 106 nodes)
SPLIT = 32768                     # int16 index split for gather
CHUNK_W = 5                       # windows per gather chunk
CW_N = 512                        # node-chunk for dense matmuls

_cache = {}


def _prep(edge_index):
    """Host-side edge bucketing -> per-core gather index / dst tables."""
    src = edge_index[0].astype(np.int64)
    dst = edge_index[1].astype(np.int64)
    core = dst // NPC
    din = dst % NPC
    w = din // WIN
    dstl = din % WIN
    is_hi = (src >= SPLIT).astype(np.int64)

    key = (core * NW + w) * 2 + is_hi
    order = np.argsort(key, kind="stable")
    s_src = src[order]
    s_dstl = dstl[order]
    s_key = key[order]
    s_hi = is_hi[order]

    counts = np.bincount(key, minlength=N_CORES * NW * 2)
    T_lo = max(1, int(-(-counts.reshape(-1, 2)[:, 0].max() // 128)))
    T_hi = max(1, int(-(-counts.reshape(-1, 2)[:, 1].max() // 128)))
    T = T_lo + T_hi

    starts = np.zeros(N_CORES * NW * 2, np.int64)
    starts[1:] = np.cumsum(counts)[:-1]
    pos = np.arange(len(s_key)) - starts[s_key]

    c_arr = s_key // (2 * NW)
    w_arr = (s_key // 2) % NW

    idx_lo = np.zeros((N_CORES, NW, T_lo * 128), np.int16)
    idx_hi = np.zeros((N_CORES, NW, T_hi * 128), np.int16)
    dstl_arr = np.full((N_CORES, NW, T, 128), -1.0, np.float16)

    lo_m = s_hi == 0
    flat = (c_arr[lo_m] * NW + w_arr[lo_m]) * (T_lo * 128) + pos[lo_m]
    idx_lo.reshape(-1)[flat] = s_src[lo_m].astype(np.int16)
    t_g = pos[lo_m] // 128
    e_g = pos[lo_m] % 128
    flat = ((c_arr[lo_m] * NW + w_arr[lo_m]) * T + t_g) * 128 + e_g
    dstl_arr.reshape(-1)[flat] = s_dstl[lo_m].astype(np.float16)

    hi_m = ~lo_m
    flat = (c_arr[hi_m] * NW + w_arr[hi_m]) * (T_hi * 128) + pos[hi_m]
    idx_hi.reshape(-1)[flat] = (s_src[hi_m] - SPLIT).astype(np.int16)
    t_g = pos[hi_m] // 128 + T_lo
    e_g = pos[hi_m] % 128
    flat = ((c_arr[hi_m] * NW + w_arr[hi_m]) * T + t_g) * 128 + e_g
    dstl_arr.reshape(-1)[flat] = s_dstl[hi_m].astype(np.float16)

    def wrap(vals):  # [NW*Tc*128] -> [128, NW*Tc*8] int16 wrapped+replicated
        v = vals.reshape(-1, 16).T  # [16, n/16]
        return np.tile(v, (8, 1)).copy()

    idx_lo_w = np.stack([wrap(idx_lo[c].reshape(-1)) for c in range(N_CORES)])
    idx_hi_w = np.stack([wrap(idx_hi[c].reshape(-1)) for c in range(N_CORES)])
    dstloc = np.ascontiguousarray(dstl_arr.transpose(0, 3, 1, 2))  # [C,128,NW,T]
    return idx_lo_w, idx_hi_w, dstloc, T_lo, T_hi


def _build(T_lo, T_hi):
    from concourse import bass, bacc, tile, mybir, library_config

    dt = mybir.dt
    T = T_lo + T_hi
    nc = bacc.Bacc("TRN2", target_bir_lowering=False, debug=False,
                   num_devices=N_CORES, num_swdge_queues=4)

    # ---- I/O ----
    xT_in = nc.dram_tensor("xT", [IN_DIM, NPC], dt.float32, kind="ExternalInput")
    w_inT_in = nc.dram_tensor("w_inT", [IN_DIM, HID], dt.float32,
                              kind="ExternalInput")
    b_in_in = nc.dram_tensor("b_in", [HID, 1], dt.float32, kind="ExternalInput")
    w_lT_in = nc.dram_tensor("w_lT", [N_LAYERS, HID, HID], dt.float32,
                             kind="ExternalInput")
    b_l_in = nc.dram_tensor("b_l", [N_LAYERS, HID, 1], dt.float32,
                            kind="ExternalInput")
    w_out4_in = nc.dram_tensor("w_out4", [N_LAYERS + 1, HID, OUT_DIM],
                               dt.float16, kind="ExternalInput")
    b_out_in = nc.dram_tensor("b_out", [OUT_DIM, 1], dt.float32,
                              kind="ExternalInput")
    iota_in = nc.dram_tensor("iota", [128, WIN], dt.float16,
                             kind="ExternalInput")
    id96_in = nc.dram_tensor("id96", [HID, HID], dt.float16,
                             kind="ExternalInput")
    id128_in = nc.dram_tensor("id128", [128, 128], dt.float32,
                              kind="ExternalInput")
    idx_lo_in = nc.dram_tensor("idx_lo", [128, NW * T_lo * 8], dt.int16,
                               kind="ExternalInput")
    idx_hi_in = nc.dram_tensor("idx_hi", [128, NW * T_hi * 8], dt.int16,
                               kind="ExternalInput")
    dstloc_in = nc.dram_tensor("dstloc", [128, NW, T], dt.float16,
                               kind="ExternalInput")
    out_ext = nc.dram_tensor("out", [NPC, OUT_DIM], dt.float32,
                             kind="ExternalOutput")

    f32, f32r, f16 = dt.float32, dt.float32r, dt.float16

    with tile.TileContext(nc, num_cores=N_CORES) as tc:
        nc.gpsimd.load_library(library_config.mlp)
        with tc.tile_pool(name="persist", bufs=1) as pp, \
             tc.tile_pool(name="xpool", bufs=3) as xpool, \
             tc.tile_pool(name="glo", bufs=2) as glo_pool, \
             tc.tile_pool(name="ghi", bufs=2) as ghi_pool, \
             tc.tile_pool(name="oh", bufs=3) as oh_pool, \
             tc.tile_pool(name="otile", bufs=2) as ot_pool, \
             tc.tile_pool(name="ps_agg", bufs=3, space="PSUM") as ps_agg, \
             tc.tile_pool(name="ps_big", bufs=2, space="PSUM") as ps_big, \
             tc.tile_pool(name="ps_tr", bufs=2, space="PSUM") as ps_tr, \
             tc.tile_pool(name="dram", bufs=1, space="DRAM") as dram:

            # ---- persistent SBUF ----
            def load(name, shape, dtype, src_ap):
                t = pp.tile(shape, dtype, name=name)
                nc.sync.dma_start(out=t[:], in_=src_ap)
                return t

            w_inT = load("w_inT", [IN_DIM, HID], f32r, w_inT_in[:].bitcast(f32r))
            b_in = load("b_in", [HID, 1], f32, b_in_in[:])
            w_lT = [load(f"w_lT{l}", [HID, HID], f32r, w_lT_in[l].bitcast(f32r))
                    for l in range(N_LAYERS)]
            b_l = [load(f"b_l{l}", [HID, 1], f32, b_l_in[l])
                   for l in range(N_LAYERS)]
            w_out4 = [load(f"w_out4_{s}", [HID, OUT_DIM], f16, w_out4_in[s])
                      for s in range(N_LAYERS + 1)]
            b_out = load("b_out", [OUT_DIM, 1], f32, b_out_in[:])
            iota = load("iota", [128, WIN], f16, iota_in[:])
            id96 = load("id96", [HID, HID], f16, id96_in[:])
            id128 = load("id128", [128, 128], f32, id128_in[:])
            idx_lo = load("idx_lo", [128, NW * T_lo * 8], dt.int16, idx_lo_in[:])
            idx_hi = load("idx_hi", [128, NW * T_hi * 8], dt.int16, idx_hi_in[:])
            dstloc = load("dstloc", [128, NW, T], f16, dstloc_in[:])

            h_state = [pp.tile([HID, NPC], f16, name=f"h{s}")
                       for s in range(N_LAYERS + 1)]
            h_plus = pp.tile([HID, NPC], f32r, name="h_plus")
            rm_buf = pp.tile([128, NW, 128], f16, name="rm_buf")

            h_full = dram.tile([N_NODES, 128], f16)
            bounce = dram.tile([NPC, 128], f16)

            node_chunks = [(j * CW_N, min(CW_N, NPC - j * CW_N))
                           for j in range(-(-NPC // CW_N))]

            def epilogue(s):
                """h_state[s] -> row-major fp16 shard -> AllGather h_full."""
                for t in range(NW):
                    n0 = t * 128
                    tn = min(128, NPC - n0)
                    pst = ps_tr.tile([128, HID], f16, name="pst")
                    nc.tensor.transpose(pst[:tn, :],
                                        h_state[s][:, n0:n0 + tn], id96[:])
                    nc.scalar.copy(rm_buf[:tn, t, 0:HID], pst[:tn, :])
                nc.sync.dma_start(
                    out=bounce[0:(NW - 1) * 128, :].rearrange(
                        "(t p) d -> p t d", p=128),
                    in_=rm_buf[:, 0:NW - 1, :])
                tn = NPC - (NW - 1) * 128
                nc.sync.dma_start(out=bounce[(NW - 1) * 128:NPC, :],
                                  in_=rm_buf[:tn, NW - 1, :])
                nc.gpsimd.collective_compute(
                    "AllGather", mybir.AluOpType.bypass,
                    ins=[bounce.opt()], outs=[h_full.opt()],
                    replica_groups=[list(range(N_CORES))])

            # ---- input projection ----
            for n0, cw in node_chunks:
                xb = xpool.tile([IN_DIM, CW_N], f32r, name="xb")
                nc.sync.dma_start(out=xb[:, :cw],
                                  in_=xT_in[:, n0:n0 + cw].bitcast(f32r))
                ps = ps_big.tile([HID, CW_N], f32, name="psb")
                nc.tensor.matmul(ps[:, :cw], w_inT[:], xb[:, :cw],
                                 start=True, stop=True)
                nc.scalar.add(h_state[0][:, n0:n0 + cw], ps[:, :cw], b_in[:])
            epilogue(0)

            # ---- GIN layers ----
            h_lo = h_full[0:SPLIT, :]
            h_hi = h_full[SPLIT:N_NODES, :]
            w_chunks = [(c0, min(CHUNK_W, NW - c0))
                        for c0 in range(0, NW, CHUNK_W)]
            for l in range(N_LAYERS):
                for ci, (c0, cw) in enumerate(w_chunks):
                    g_lo = glo_pool.tile([128, CHUNK_W * T_lo, 128], f16,
                                         name="g_lo")
                    nc.gpsimd.dma_gather(
                        g_lo[:, :cw * T_lo, :], h_lo,
                        idx_lo[:, c0 * T_lo * 8:(c0 + cw) * T_lo * 8],
                        num_idxs=cw * T_lo * 128,
                        num_idxs_reg=cw * T_lo * 128, elem_size=128,
                        single_packet=False, queue_num=(2 * ci) % 4)
                    g_hi = ghi_pool.tile([128, CHUNK_W * T_hi, 128], f16,
                                         name="g_hi")
                    nc.gpsimd.dma_gather(
                        g_hi[:, :cw * T_hi, :], h_hi,
                        idx_hi[:, c0 * T_hi * 8:(c0 + cw) * T_hi * 8],
                        num_idxs=cw * T_hi * 128,
                        num_idxs_reg=cw * T_hi * 128, elem_size=128,
                        single_packet=False, queue_num=(2 * ci + 1) % 4)
                    for wl in range(cw):
                        w_i = c0 + wl
                        n0 = w_i * 128
                        wn = min(128, NPC - n0)
                        oh = oh_pool.tile([128, T, WIN], f16, name="oh")
                        nc.vector.tensor_tensor(
                            oh[:],
                            iota[:].unsqueeze(1).broadcast_to([128, T, WIN]),
                            dstloc[:, w_i, :].unsqueeze(2)
                                .broadcast_to([128, T, WIN]),
                            mybir.AluOpType.is_equal)
                        ps = ps_agg.tile([HID, WIN], f32, name="psa")
                        for t in range(T):
                            if t < T_lo:
                                lhsT = g_lo[:, wl * T_lo + t, 0:HID]
                            else:
                                lhsT = g_hi[:, wl * T_hi + (t - T_lo), 0:HID]
                            nc.tensor.matmul(ps[:], lhsT, oh[:, t, :],
                                             start=(t == 0),
                                             stop=(t == T - 1))
                        nc.vector.tensor_tensor(
                            h_plus[:, n0:n0 + wn], ps[:, :wn],
                            h_state[l][:, n0:n0 + wn], mybir.AluOpType.add)
                for n0, cw in node_chunks:
                    ps = ps_big.tile([HID, CW_N], f32, name="psb")
                    nc.tensor.matmul(ps[:, :cw], w_lT[l][:],
                                     h_plus[:, n0:n0 + cw],
                                     start=True, stop=True)
                    nc.scalar.add(h_state[l + 1][:, n0:n0 + cw], ps[:, :cw],
                                  b_l[l][:])
                if l < N_LAYERS - 1:
                    epilogue(l + 1)

            # ---- output projection ----
            for n0, cw in node_chunks:
                ps = ps_big.tile([OUT_DIM, CW_N], f32, name="pso", tag="psb")
                for s in range(N_LAYERS + 1):
                    nc.tensor.matmul(ps[:, :cw], w_out4[s][:],
                                     h_state[s][:, n0:n0 + cw],
                                     start=(s == 0), stop=(s == N_LAYERS))
                ot = ot_pool.tile([OUT_DIM, CW_N], f32, name="ot")
                nc.scalar.add(ot[:, :cw], ps[:, :cw], b_out[:])
                for tt in range(-(-cw // 128)):
                    t0 = tt * 128
                    tn = min(128, cw - t0)
                    pst = ps_tr.tile([128, 128], f32, name="psto", tag="pst")
                    nc.tensor.transpose(pst[:tn, :], ot[:, t0:t0 + tn],
                                        id128[:])
                    orow = ot_pool.tile([128, 128], f32, name="orow")
                    nc.scalar.copy(orow[:tn, :], pst[:tn, :])
                    nc.sync.dma_start(
                        out=out_ext[n0 + t0:n0 + t0 + tn, :],
                        in_=orow[:tn, :])

    nc.compile()
    return nc


def _get_nc_and_inputs(inputs):
    from concourse import bass_utils  # noqa: F401  (path setup)

    x = np.asarray(inputs["x"], np.float32)
    edge_index = np.asarray(inputs["edge_index"], np.int32)
    W_in = np.asarray(inputs["W_in"], np.float32)
    b_in = np.asarray(inputs["b_in"], np.float32)
    W_layers = np.asarray(inputs["W_layers"], np.float32)
    b_layers = np.asarray(inputs["b_layers"], np.float32)
    W_out = np.asarray(inputs["W_out"], np.float32)
    b_out = np.asarray(inputs["b_out"], np.float32)

    idx_lo_w, idx_hi_w, dstloc, T_lo, T_hi = _prep(edge_index)

    key = ("nc", T_lo, T_hi)
    if key not in _cache:
        _cache.clear()
        _cache[key] = _build(T_lo, T_hi)
    nc = _cache[key]

    xT = np.ascontiguousarray(x.T)
    w_inT = np.ascontiguousarray(W_in.T)
    w_lT = np.ascontiguousarray(W_layers.transpose(0, 2, 1))
    b_l = np.ascontiguousarray(b_layers[:, :, None])
    w_out4 = np.ascontiguousarray(
        np.stack([W_out[:, s * HID:(s + 1) * HID].T
                  for s in range(N_LAYERS + 1)])).astype(np.float16)
    iota = np.tile(np.arange(WIN, dtype=np.float16), (128, 1))
    id96 = np.eye(HID, dtype=np.float16)
    id128 = np.eye(128, dtype=np.float32)

    in_maps = []
    for c in range(N_CORES):
        in_maps.append({
            "xT": np.ascontiguousarray(xT[:, c * NPC:(c + 1) * NPC]),
            "w_inT": w_inT,
            "b_in": b_in.reshape(HID, 1),
            "w_lT": w_lT,
            "b_l": b_l,
            "w_out4": w_out4,
            "b_out": b_out.reshape(OUT_DIM, 1),
            "iota": iota,
            "id96": id96,
            "id128": id128,
            "idx_lo": idx_lo_w[c],
            "idx_hi": idx_hi_w[c],
            "dstloc": dstloc[c],
        })
    return nc, in_maps


def run(inputs, trace=False):
    from concourse import bass_utils

    nc, in_maps = _get_nc_and_inputs(inputs)
    res = bass_utils.run_bass_kernel_spmd(
        nc, in_maps, core_ids=list(range(N_CORES)), trace=trace)
    out = np.concatenate([res.results[c]["out"] for c in range(N_CORES)], 0)
    return out, res


def kernel(**inputs):
    out, _ = run(inputs, trace=False)
    return out


# revision 9
# speedup vs baseline: 1.4171x; 1.4055x over previous
"""GIN-style 3-layer GNN encoder on 8 Trainium2 NeuronCores (Bass/Tile).

Reference computation (fp32):
    h = x @ W_in.T + b_in                                  [50000, 96]
    for l in 0..2:
        agg = segment_sum(h[src], dst, N)                  [50000, 96]
        h = (h + agg) @ W_layers[l].T + b_layers[l]
    out = concat([h0..h3], 1) @ W_out.T + b_out            [50000, 128]

Distribution: nodes are partitioned contiguously across the 8 cores
(6250/core); each edge is owned by the core that owns its dst node.  Each
layer, the updated node features are AllGathered into a replicated
row-major fp16 table h_full [50000, 128pad] in each core's HBM.

Per-core segment sum: a core's node range is split into 49 windows of 128
nodes.  Every window gets a fixed number of 128-edge tiles (T_lo tiles
whose src < 32768, T_hi tiles with src >= 32768 — the split keeps gather
indices inside int16 range; host pads with dummy edges src=0 / dst=-1).
Edge features are fetched with gpsimd dma_gather (fp16 256B rows, edge i
of a gather lands in partition i%128).  For each window the one-hot
matrix onehot[e, t, j] = (dst_local[e, t] == j) is built on DVE with a
single broadcast is_equal per window, and the PE computes
    psum[96, 128] += gathered_tile[128e, 96].T @ onehot_tile[128e, 128]
which is exactly aggT for the window.  h+agg, the layer matmul (fp32r),
bias (ACT) follow; a PE transpose produces the row-major fp16 shard for
the next AllGather.
"""
import sys

sys.path.insert(0, "/opt/trn_rl_repo")

import numpy as np

N_NODES = 50000
N_EDGES = 800000
IN_DIM = 128
HID = 96
OUT_DIM = 128
N_LAYERS = 3
N_CORES = 8
NPC = N_NODES // N_CORES          # 6250 nodes per core
WIN = 128                         # window width (nodes)
NW = (NPC + WIN - 1) // WIN       # 49 windows per core (last = 106 nodes)
SPLIT = 32768                     # int16 index split for gather
CHUNK_W = 5                       # windows per gather chunk
CW_N = 512                        # node-chunk for dense matmuls

_cache = {}


def _prep(edge_index):
    """Host-side edge bucketing -> per-core gather index / dst tables."""
    src = edge_index[0].astype(np.int64)
    dst = edge_index[1].astype(np.int64)
    core = dst // NPC
    din = dst % NPC
    w = din // WIN
    dstl = din % WIN
    is_hi = (src >= SPLIT).astype(np.int64)

    key = (core * NW + w) * 2 + is_hi
    order = np.argsort(key, kind="stable")
    s_src = src[order]
    s_dstl = dstl[order]
    s_key = key[order]
    s_hi = is_hi[order]

    counts = np.bincount(key, minlength=N_CORES * NW * 2)
    T_lo = max(1, int(-(-counts.reshape(-1, 2)[:, 0].max() // 128)))
    T_hi = max(1, int(-(-counts.reshape(-1, 2)[:, 1].max() // 128)))
    T = T_lo + T_hi

    starts = np.zeros(N_CORES * NW * 2, np.int64)
    starts[1:] = np.cumsum(counts)[:-1]
    pos = np.arange(len(s_key)) - starts[s_key]

    c_arr = s_key // (2 * NW)
    w_arr = (s_key // 2) % NW

    idx_lo = np.zeros((N_CORES, NW, T_lo * 128), np.int16)
    idx_hi = np.zeros((N_CORES, NW, T_hi * 128), np.int16)
    dstl_arr = np.full((N_CORES, NW, T, 128), -1.0, np.float16)

    lo_m = s_hi == 0
    flat = (c_arr[lo_m] * NW + w_arr[lo_m]) * (T_lo * 128) + pos[lo_m]
    idx_lo.reshape(-1)[flat] = s_src[lo_m].astype(np.int16)
    t_g = pos[lo_m] // 128
    e_g = pos[lo_m] % 128
    flat = ((c_arr[lo_m] * NW + w_arr[lo_m]) * T + t_g) * 128 + e_g
    dstl_arr.reshape(-1)[flat] = s_dstl[lo_m].astype(np.float16)

    hi_m = ~lo_m
    flat = (c_arr[hi_m] * NW + w_arr[hi_m]) * (T_hi * 128) + pos[hi_m]
    idx_hi.reshape(-1)[flat] = (s_src[hi_m] - SPLIT).astype(np.int16)
    t_g = pos[hi_m] // 128 + T_lo
    e_g = pos[hi_m] % 128
    flat = ((c_arr[hi_m] * NW + w_arr[hi_m]) * T + t_g) * 128 + e_g
    dstl_arr.reshape(-1)[flat] = s_dstl[hi_m].astype(np.float16)

    def wrap(vals):  # [NW*Tc*128] -> [128, NW*Tc*8] int16 wrapped+replicated
        v = vals.reshape(-1, 16).T  # [16, n/16]
        return np.tile(v, (8, 1)).copy()

    idx_lo_w = np.stack([wrap(idx_lo[c].reshape(-1)) for c in range(N_CORES)])
    idx_hi_w = np.stack([wrap(idx_hi[c].reshape(-1)) for c in range(N_CORES)])
    dstloc = np.ascontiguousarray(dstl_arr.transpose(0, 3, 1, 2))  # [C,128,NW,T]
    return idx_lo_w, idx_hi_w, dstloc, T_lo, T_hi


def _build(T_lo, T_hi):
    from concourse import bass, bacc, tile, mybir, library_config

    dt = mybir.dt
    T = T_lo + T_hi
    nc = bacc.Bacc("TRN2", target_bir_lowering=False, debug=False,
                   num_devices=N_CORES, num_swdge_queues=4)

    # ---- I/O ----
    xT_in = nc.dram_tensor("xT", [IN_DIM, NPC], dt.float32, kind="ExternalInput")
    w_inT_in = nc.dram_tensor("w_inT", [IN_DIM, HID], dt.float32,
                              kind="ExternalInput")
    b_in_in = nc.dram_tensor("b_in", [HID, 1], dt.float32, kind="ExternalInput")
    w_lT_in = nc.dram_tensor("w_lT", [N_LAYERS, HID, HID], dt.float32,
                             kind="ExternalInput")
    b_l_in = nc.dram_tensor("b_l", [N_LAYERS, HID, 1], dt.float32,
                            kind="ExternalInput")
    w_out4_in = nc.dram_tensor("w_out4", [N_LAYERS + 1, HID, OUT_DIM],
                               dt.float16, kind="ExternalInput")
    b_out_in = nc.dram_tensor("b_out", [OUT_DIM, 1], dt.float32,
                              kind="ExternalInput")
    iota_in = nc.dram_tensor("iota", [128, WIN], dt.float16,
                             kind="ExternalInput")
    id96_in = nc.dram_tensor("id96", [HID, HID], dt.float16,
                             kind="ExternalInput")
    id128_in = nc.dram_tensor("id128", [128, 128], dt.float32,
                              kind="ExternalInput")
    idx_lo_in = nc.dram_tensor("idx_lo", [128, NW * T_lo * 8], dt.int16,
                               kind="ExternalInput")
    idx_hi_in = nc.dram_tensor("idx_hi", [128, NW * T_hi * 8], dt.int16,
                               kind="ExternalInput")
    dstloc_in = nc.dram_tensor("dstloc", [128, NW, T], dt.float16,
                               kind="ExternalInput")
    out_ext = nc.dram_tensor("out", [NPC, OUT_DIM], dt.float32,
                             kind="ExternalOutput")

    f32, f32r, f16 = dt.float32, dt.float32r, dt.float16

    with tile.TileContext(nc, num_cores=N_CORES) as tc:
        nc.gpsimd.load_library(library_config.mlp)
        with tc.tile_pool(name="persist", bufs=1) as pp, \
             tc.tile_pool(name="xpool", bufs=3) as xpool, \
             tc.tile_pool(name="glo", bufs=2) as glo_pool, \
             tc.tile_pool(name="ghi", bufs=2) as ghi_pool, \
             tc.tile_pool(name="oh", bufs=3) as oh_pool, \
             tc.tile_pool(name="otile", bufs=2) as ot_pool, \
             tc.tile_pool(name="ps_agg", bufs=3, space="PSUM") as ps_agg, \
             tc.tile_pool(name="ps_big", bufs=2, space="PSUM") as ps_big, \
             tc.tile_pool(name="ps_tr", bufs=2, space="PSUM") as ps_tr, \
             tc.tile_pool(name="dram", bufs=1, space="DRAM") as dram:

            # ---- persistent SBUF ----
            def load(name, shape, dtype, src_ap):
                t = pp.tile(shape, dtype, name=name)
                nc.sync.dma_start(out=t[:], in_=src_ap)
                return t

            w_inT = load("w_inT", [IN_DIM, HID], f32r, w_inT_in[:].bitcast(f32r))
            b_in = load("b_in", [HID, 1], f32, b_in_in[:])
            w_lT = [load(f"w_lT{l}", [HID, HID], f32r, w_lT_in[l].bitcast(f32r))
                    for l in range(N_LAYERS)]
            b_l = [load(f"b_l{l}", [HID, 1], f32, b_l_in[l])
                   for l in range(N_LAYERS)]
            w_out4 = [load(f"w_out4_{s}", [HID, OUT_DIM], f16, w_out4_in[s])
                      for s in range(N_LAYERS + 1)]
            b_out = load("b_out", [OUT_DIM, 1], f32, b_out_in[:])
            iota = load("iota", [128, WIN], f16, iota_in[:])
            id96 = load("id96", [HID, HID], f16, id96_in[:])
            id128 = load("id128", [128, 128], f32, id128_in[:])
            idx_lo = load("idx_lo", [128, NW * T_lo * 8], dt.int16, idx_lo_in[:])
            idx_hi = load("idx_hi", [128, NW * T_hi * 8], dt.int16, idx_hi_in[:])
            dstloc = load("dstloc", [128, NW, T], f16, dstloc_in[:])

            h_state = [pp.tile([HID, NPC], f16, name=f"h{s}")
                       for s in range(N_LAYERS + 1)]
            h_plus = pp.tile([HID, NPC], f32r, name="h_plus")
            rm_buf = pp.tile([128, NW, 128], f16, name="rm_buf")

            h_full = dram.tile([N_NODES, 128], f16)
            bounce = dram.tile([NPC, 128], f16)

            node_chunks = [(j * CW_N, min(CW_N, NPC - j * CW_N))
                           for j in range(-(-NPC // CW_N))]

            def epilogue(s):
                """h_state[s] -> row-major fp16 shard -> AllGather h_full."""
                for t in range(NW):
                    n0 = t * 128
                    tn = min(128, NPC - n0)
                    pst = ps_tr.tile([128, HID], f16, name="pst")
                    nc.tensor.transpose(pst[:tn, :],
                                        h_state[s][:, n0:n0 + tn], id96[:])
                    nc.scalar.copy(rm_buf[:tn, t, 0:HID], pst[:tn, :])
                nc.sync.dma_start(
                    out=bounce[0:(NW - 1) * 128, :].rearrange(
                        "(t p) d -> p t d", p=128),
                    in_=rm_buf[:, 0:NW - 1, :])
                tn = NPC - (NW - 1) * 128
                nc.sync.dma_start(out=bounce[(NW - 1) * 128:NPC, :],
                                  in_=rm_buf[:tn, NW - 1, :])
                nc.gpsimd.collective_compute(
                    "AllGather", mybir.AluOpType.bypass,
                    ins=[bounce.opt()], outs=[h_full.opt()],
                    replica_groups=[list(range(N_CORES))])

            # ---- input projection ----
            for n0, cw in node_chunks:
                xb = xpool.tile([IN_DIM, CW_N], f32r, name="xb")
                nc.sync.dma_start(out=xb[:, :cw],
                                  in_=xT_in[:, n0:n0 + cw].bitcast(f32r))
                ps = ps_big.tile([HID, CW_N], f32, name="psb")
                nc.tensor.matmul(ps[:, :cw], w_inT[:], xb[:, :cw],
                                 start=True, stop=True)
                nc.scalar.add(h_state[0][:, n0:n0 + cw], ps[:, :cw], b_in[:])
            epilogue(0)

            # ---- GIN layers ----
            h_lo = h_full[0:SPLIT, :]
            h_hi = h_full[SPLIT:N_NODES, :]
            w_chunks = [(c0, min(CHUNK_W, NW - c0))
                        for c0 in range(0, NW, CHUNK_W)]
            # gather unit: GT tiles (GT*128 idxs <= 1024 so the 64
            # descriptors per SDMA engine fit one packet), round-robin
            # over the 4 SWDGE queues (4 Q7 pairs generate + 4 rings
            # feed each engine concurrently)
            GT = 8
            qrr = [0]

            def emit_gathers(gbuf, src_view, idx_tile, base_tile, n_tiles):
                for s0 in range(0, n_tiles, GT):
                    sn = min(GT, n_tiles - s0)
                    nc.gpsimd.dma_gather(
                        gbuf[:, s0:s0 + sn, :], src_view,
                        idx_tile[:, (base_tile + s0) * 8:
                                 (base_tile + s0 + sn) * 8],
                        num_idxs=sn * 128, num_idxs_reg=sn * 128,
                        elem_size=128, single_packet=True,
                        queue_num=qrr[0] % 4)
                    qrr[0] += 1

            for l in range(N_LAYERS):
                for ci, (c0, cw) in enumerate(w_chunks):
                    g_lo = glo_pool.tile([128, CHUNK_W * T_lo, 128], f16,
                                         name="g_lo")
                    emit_gathers(g_lo, h_lo, idx_lo, c0 * T_lo, cw * T_lo)
                    g_hi = ghi_pool.tile([128, CHUNK_W * T_hi, 128], f16,
                                         name="g_hi")
                    emit_gathers(g_hi, h_hi, idx_hi, c0 * T_hi, cw * T_hi)
                    for wl in range(cw):
                        w_i = c0 + wl
                        n0 = w_i * 128
                        wn = min(128, NPC - n0)
                        oh = oh_pool.tile([128, T, WIN], f16, name="oh")
                        nc.vector.tensor_tensor(
                            oh[:],
                            iota[:].unsqueeze(1).broadcast_to([128, T, WIN]),
                            dstloc[:, w_i, :].unsqueeze(2)
                                .broadcast_to([128, T, WIN]),
                            mybir.AluOpType.is_equal)
                        ps = ps_agg.tile([HID, WIN], f32, name="psa")
                        for t in range(T):
                            if t < T_lo:
                                lhsT = g_lo[:, wl * T_lo + t, 0:HID]
                            else:
                                lhsT = g_hi[:, wl * T_hi + (t - T_lo), 0:HID]
                            nc.tensor.matmul(ps[:], lhsT, oh[:, t, :],
                                             start=(t == 0),
                                             stop=(t == T - 1))
                        nc.vector.tensor_tensor(
                            h_plus[:, n0:n0 + wn], ps[:, :wn],
                            h_state[l][:, n0:n0 + wn], mybir.AluOpType.add)
                for n0, cw in node_chunks:
                    ps = ps_big.tile([HID, CW_N], f32, name="psb")
                    nc.tensor.matmul(ps[:, :cw], w_lT[l][:],
                                     h_plus[:, n0:n0 + cw],
                                     start=True, stop=True)
                    nc.scalar.add(h_state[l + 1][:, n0:n0 + cw], ps[:, :cw],
                                  b_l[l][:])
                if l < N_LAYERS - 1:
                    epilogue(l + 1)

            # ---- output projection ----
            for n0, cw in node_chunks:
                ps = ps_big.tile([OUT_DIM, CW_N], f32, name="pso", tag="psb")
                for s in range(N_LAYERS + 1):
                    nc.tensor.matmul(ps[:, :cw], w_out4[s][:],
                                     h_state[s][:, n0:n0 + cw],
                                     start=(s == 0), stop=(s == N_LAYERS))
                ot = ot_pool.tile([OUT_DIM, CW_N], f32, name="ot")
                nc.scalar.add(ot[:, :cw], ps[:, :cw], b_out[:])
                for tt in range(-(-cw // 128)):
                    t0 = tt * 128
                    tn = min(128, cw - t0)
                    pst = ps_tr.tile([128, 128], f32, name="psto", tag="pst")
                    nc.tensor.transpose(pst[:tn, :], ot[:, t0:t0 + tn],
                                        id128[:])
                    orow = ot_pool.tile([128, 128], f32, name="orow")
                    nc.scalar.copy(orow[:tn, :], pst[:tn, :])
                    nc.sync.dma_start(
                        out=out_ext[n0 + t0:n0 + t0 + tn, :],
                        in_=orow[:tn, :])

    nc.compile()
    return nc


def _get_nc_and_inputs(inputs):
    from concourse import bass_utils  # noqa: F401  (path setup)

    x = np.asarray(inputs["x"], np.float32)
    edge_index = np.asarray(inputs["edge_index"], np.int32)
    W_in = np.asarray(inputs["W_in"], np.float32)
    b_in = np.asarray(inputs["b_in"], np.float32)
    W_layers = np.asarray(inputs["W_layers"], np.float32)
    b_layers = np.asarray(inputs["b_layers"], np.float32)
    W_out = np.asarray(inputs["W_out"], np.float32)
    b_out = np.asarray(inputs["b_out"], np.float32)

    idx_lo_w, idx_hi_w, dstloc, T_lo, T_hi = _prep(edge_index)

    key = ("nc", T_lo, T_hi)
    if key not in _cache:
        _cache.clear()
        _cache[key] = _build(T_lo, T_hi)
    nc = _cache[key]

    xT = np.ascontiguousarray(x.T)
    w_inT = np.ascontiguousarray(W_in.T)
    w_lT = np.ascontiguousarray(W_layers.transpose(0, 2, 1))
    b_l = np.ascontiguousarray(b_layers[:, :, None])
    w_out4 = np.ascontiguousarray(
        np.stack([W_out[:, s * HID:(s + 1) * HID].T
                  for s in range(N_LAYERS + 1)])).astype(np.float16)
    iota = np.tile(np.arange(WIN, dtype=np.float16), (128, 1))
    id96 = np.eye(HID, dtype=np.float16)
    id128 = np.eye(128, dtype=np.float32)

    in_maps = []
    for c in range(N_CORES):
        in_maps.append({
            "xT": np.ascontiguousarray(xT[:, c * NPC:(c + 1) * NPC]),
            "w_inT": w_inT,
            "b_in": b_in.reshape(HID, 1),
            "w_lT": w_lT,
            "b_l": b_l,
            "w_out4": w_out4,
            "b_out": b_out.reshape(OUT_DIM, 1),
            "iota": iota,
            "id96": id96,
            "id128": id128,
            "idx_lo": idx_lo_w[c],
            "idx_hi": idx_hi_w[c],
            "dstloc": dstloc[c],
        })
    return nc, in_maps


def run(inputs, trace=False):
    from concourse import bass_utils

    nc, in_maps = _get_nc_and_inputs(inputs)
    res = bass_utils.run_bass_kernel_spmd(
        nc, in_maps, core_ids=list(range(N_CORES)), trace=trace)
    out = np.concatenate([res.results[c]["out"] for c in range(N_CORES)], 0)
    return out, res


def kernel(**inputs):
    out, _ = run(inputs, trace=False)
    return out


# revision 12
# speedup vs baseline: 2.3941x; 1.6895x over previous
"""GIN-style 3-layer GNN encoder on 8 Trainium2 NeuronCores (Bass/Tile).

Reference computation (fp32):
    h = x @ W_in.T + b_in                                  [50000, 96]
    for l in 0..2:
        agg = segment_sum(h[src], dst, N)                  [50000, 96]
        h = (h + agg) @ W_layers[l].T + b_layers[l]
    out = concat([h0..h3], 1) @ W_out.T + b_out            [50000, 128]

Distribution: nodes are partitioned contiguously across the 8 cores
(6250/core); each edge is owned by the core that owns its dst node.  Each
layer, the updated node features are AllGathered into a replicated
row-major fp16 table h_full [50000, 128pad] in each core's HBM.

Per-core segment sum: a core's node range is split into 49 windows of 128
nodes.  Every window gets a fixed number of 128-edge tiles (T_lo tiles
whose src < 32768, T_hi tiles with src >= 32768 — the split keeps gather
indices inside int16 range; host pads with dummy edges src=0 / dst=-1).
Edge features are fetched with gpsimd dma_gather (fp16 256B rows, edge i
of a gather lands in partition i%128).  For each window the one-hot
matrix onehot[e, t, j] = (dst_local[e, t] == j) is built on DVE with a
single broadcast is_equal per window, and the PE computes
    psum[96, 128] += gathered_tile[128e, 96].T @ onehot_tile[128e, 128]
which is exactly aggT for the window.  h+agg, the layer matmul (fp32r),
bias (ACT) follow; a PE transpose produces the row-major fp16 shard for
the next AllGather.
"""
import sys

sys.path.insert(0, "/opt/trn_rl_repo")

import numpy as np

N_NODES = 50000
N_EDGES = 800000
IN_DIM = 128
HID = 96
OUT_DIM = 128
N_LAYERS = 3
N_CORES = 8
NPC = N_NODES // N_CORES          # 6250 nodes per core
WIN = 128                         # window width (nodes)
NW = (NPC + WIN - 1) // WIN       # 49 windows per core (last = 106 nodes)
SPLIT = 32768                     # int16 index split for gather
CHUNK_W = 5                       # windows per gather chunk
CW_N = 512                        # node-chunk for dense matmuls

_cache = {}


def _balance_nodes(src, dst):
    """Permute node ids so per-(core,window) lo/hi edge counts are even.

    A node's lo/hi class (gather-index split at id 32768) is frozen to its
    OLD id; the permutation only moves nodes within their class region, so
    per-node (deg_lo, deg_hi) are fixed and a greedy 2-D bin balance over
    the 392 (core, window) bins makes the uniform tile counts T_lo/T_hi
    tight (less gather padding).  Returns perm (old id -> new id).
    """
    deg_lo = np.bincount(dst[src < SPLIT], minlength=N_NODES).astype(np.int64)
    deg_hi = np.bincount(dst[src >= SPLIT], minlength=N_NODES).astype(np.int64)
    nbins = N_CORES * NW
    base = np.empty(nbins, np.int64)
    cap = np.empty(nbins, np.int64)
    for b in range(nbins):
        c, w = divmod(b, NW)
        base[b] = c * NPC + w * WIN
        cap[b] = min(WIN, NPC - w * WIN)
    q_lo = np.maximum(0, np.minimum(cap, SPLIT - base))  # lo slots per bin
    q_hi = cap - q_lo

    mu_lo = max(1.0, deg_lo.sum() / nbins)
    mu_hi = max(1.0, deg_hi.sum() / nbins)
    order = np.argsort(-(deg_lo + deg_hi), kind="stable")
    lo_load = np.zeros(nbins)
    hi_load = np.zeros(nbins)
    lo_left = q_lo.copy()
    hi_left = q_hi.copy()
    lo_pos = np.zeros(nbins, np.int64)
    hi_pos = q_lo.copy()  # hi slots follow the lo slots within a bin
    perm = np.empty(N_NODES, np.int64)
    inf = np.inf
    for n in order:
        phi = np.maximum((lo_load + deg_lo[n]) / mu_lo,
                         (hi_load + deg_hi[n]) / mu_hi)
        if n < SPLIT:
            phi = np.where(lo_left > 0, phi, inf)
            b = int(np.argmin(phi))
            perm[n] = base[b] + lo_pos[b]
            lo_pos[b] += 1
            lo_left[b] -= 1
        else:
            phi = np.where(hi_left > 0, phi, inf)
            b = int(np.argmin(phi))
            perm[n] = base[b] + hi_pos[b]
            hi_pos[b] += 1
            hi_left[b] -= 1
        lo_load[b] += deg_lo[n]
        hi_load[b] += deg_hi[n]
    return perm


def _prep(edge_index):
    """Host-side edge bucketing -> per-core gather index / dst tables."""
    src0 = edge_index[0].astype(np.int64)
    dst0 = edge_index[1].astype(np.int64)
    perm = _balance_nodes(src0, dst0)
    src = perm[src0]
    dst = perm[dst0]
    core = dst // NPC
    din = dst % NPC
    w = din // WIN
    dstl = din % WIN
    is_hi = (src >= SPLIT).astype(np.int64)

    key = (core * NW + w) * 2 + is_hi
    order = np.argsort(key, kind="stable")
    s_src = src[order]
    s_dstl = dstl[order]
    s_key = key[order]
    s_hi = is_hi[order]

    counts = np.bincount(key, minlength=N_CORES * NW * 2)
    T_lo = max(1, int(-(-counts.reshape(-1, 2)[:, 0].max() // 128)))
    T_hi = max(1, int(-(-counts.reshape(-1, 2)[:, 1].max() // 128)))
    T = T_lo + T_hi

    starts = np.zeros(N_CORES * NW * 2, np.int64)
    starts[1:] = np.cumsum(counts)[:-1]
    pos = np.arange(len(s_key)) - starts[s_key]

    c_arr = s_key // (2 * NW)
    w_arr = (s_key // 2) % NW

    idx_lo = np.zeros((N_CORES, NW, T_lo * 128), np.int16)
    idx_hi = np.zeros((N_CORES, NW, T_hi * 128), np.int16)
    dstl_arr = np.full((N_CORES, NW, T, 128), -1.0, np.float16)

    lo_m = s_hi == 0
    flat = (c_arr[lo_m] * NW + w_arr[lo_m]) * (T_lo * 128) + pos[lo_m]
    idx_lo.reshape(-1)[flat] = s_src[lo_m].astype(np.int16)
    t_g = pos[lo_m] // 128
    e_g = pos[lo_m] % 128
    flat = ((c_arr[lo_m] * NW + w_arr[lo_m]) * T + t_g) * 128 + e_g
    dstl_arr.reshape(-1)[flat] = s_dstl[lo_m].astype(np.float16)

    hi_m = ~lo_m
    flat = (c_arr[hi_m] * NW + w_arr[hi_m]) * (T_hi * 128) + pos[hi_m]
    idx_hi.reshape(-1)[flat] = (s_src[hi_m] - SPLIT).astype(np.int16)
    t_g = pos[hi_m] // 128 + T_lo
    e_g = pos[hi_m] % 128
    flat = ((c_arr[hi_m] * NW + w_arr[hi_m]) * T + t_g) * 128 + e_g
    dstl_arr.reshape(-1)[flat] = s_dstl[hi_m].astype(np.float16)

    def wrap(vals):  # [NW*Tc*128] -> [128, NW*Tc*8] int16 wrapped+replicated
        v = vals.reshape(-1, 16).T  # [16, n/16]
        return np.tile(v, (8, 1)).copy()

    idx_lo_w = np.stack([wrap(idx_lo[c].reshape(-1)) for c in range(N_CORES)])
    idx_hi_w = np.stack([wrap(idx_hi[c].reshape(-1)) for c in range(N_CORES)])
    dstloc = np.ascontiguousarray(dstl_arr.transpose(0, 3, 1, 2))  # [C,128,NW,T]
    return idx_lo_w, idx_hi_w, dstloc, T_lo, T_hi, perm


def _build(T_lo, T_hi):
    from concourse import bass, bacc, tile, mybir, library_config

    dt = mybir.dt
    T = T_lo + T_hi
    nc = bacc.Bacc("TRN2", target_bir_lowering=False, debug=False,
                   num_devices=N_CORES, num_swdge_queues=4)

    # ---- I/O ----
    xT_in = nc.dram_tensor("xT", [IN_DIM, NPC], dt.float32, kind="ExternalInput")
    w_inT_in = nc.dram_tensor("w_inT", [IN_DIM, HID], dt.float32,
                              kind="ExternalInput")
    b_in_in = nc.dram_tensor("b_in", [HID, 1], dt.float32, kind="ExternalInput")
    w_lT_in = nc.dram_tensor("w_lT", [N_LAYERS, HID, HID], dt.float32,
                             kind="ExternalInput")
    b_l_in = nc.dram_tensor("b_l", [N_LAYERS, HID, 1], dt.float32,
                            kind="ExternalInput")
    w_out4_in = nc.dram_tensor("w_out4", [N_LAYERS + 1, HID, OUT_DIM],
                               dt.float16, kind="ExternalInput")
    b_out_in = nc.dram_tensor("b_out", [OUT_DIM, 1], dt.float32,
                              kind="ExternalInput")
    iota_in = nc.dram_tensor("iota", [128, WIN], dt.float16,
                             kind="ExternalInput")
    id96_in = nc.dram_tensor("id96", [HID, HID], dt.float16,
                             kind="ExternalInput")
    id128_in = nc.dram_tensor("id128", [128, 128], dt.float32,
                              kind="ExternalInput")
    idx_lo_in = nc.dram_tensor("idx_lo", [128, NW * T_lo * 8], dt.int16,
                               kind="ExternalInput")
    idx_hi_in = nc.dram_tensor("idx_hi", [128, NW * T_hi * 8], dt.int16,
                               kind="ExternalInput")
    dstloc_in = nc.dram_tensor("dstloc", [128, NW, T], dt.float16,
                               kind="ExternalInput")
    out_ext = nc.dram_tensor("out", [NPC, OUT_DIM], dt.float32,
                             kind="ExternalOutput")

    f32, f32r, f16 = dt.float32, dt.float32r, dt.float16

    with tile.TileContext(nc, num_cores=N_CORES) as tc:
        nc.gpsimd.load_library(library_config.mlp)
        with tc.tile_pool(name="persist", bufs=1) as pp, \
             tc.tile_pool(name="xpool", bufs=3) as xpool, \
             tc.tile_pool(name="glo", bufs=2) as glo_pool, \
             tc.tile_pool(name="ghi", bufs=2) as ghi_pool, \
             tc.tile_pool(name="oh", bufs=3) as oh_pool, \
             tc.tile_pool(name="otile", bufs=2) as ot_pool, \
             tc.tile_pool(name="ps_agg", bufs=3, space="PSUM") as ps_agg, \
             tc.tile_pool(name="ps_big", bufs=2, space="PSUM") as ps_big, \
             tc.tile_pool(name="ps_tr", bufs=2, space="PSUM") as ps_tr, \
             tc.tile_pool(name="dram", bufs=1, space="DRAM") as dram:

            # ---- persistent SBUF ----
            def load(name, shape, dtype, src_ap):
                t = pp.tile(shape, dtype, name=name)
                nc.sync.dma_start(out=t[:], in_=src_ap)
                return t

            w_inT = load("w_inT", [IN_DIM, HID], f32r, w_inT_in[:].bitcast(f32r))
            b_in = load("b_in", [HID, 1], f32, b_in_in[:])
            w_lT = [load(f"w_lT{l}", [HID, HID], f32r, w_lT_in[l].bitcast(f32r))
                    for l in range(N_LAYERS)]
            b_l = [load(f"b_l{l}", [HID, 1], f32, b_l_in[l])
                   for l in range(N_LAYERS)]
            w_out4 = [load(f"w_out4_{s}", [HID, OUT_DIM], f16, w_out4_in[s])
                      for s in range(N_LAYERS + 1)]
            b_out = load("b_out", [OUT_DIM, 1], f32, b_out_in[:])
            iota = load("iota", [128, WIN], f16, iota_in[:])
            id96 = load("id96", [HID, HID], f16, id96_in[:])
            id128 = load("id128", [128, 128], f32, id128_in[:])
            idx_lo = load("idx_lo", [128, NW * T_lo * 8], dt.int16, idx_lo_in[:])
            idx_hi = load("idx_hi", [128, NW * T_hi * 8], dt.int16, idx_hi_in[:])
            dstloc = load("dstloc", [128, NW, T], f16, dstloc_in[:])

            h_state = [pp.tile([HID, NPC], f16, name=f"h{s}")
                       for s in range(N_LAYERS + 1)]
            h_plus = pp.tile([HID, NPC], f32r, name="h_plus")
            rm_buf = pp.tile([128, NW, 128], f16, name="rm_buf")

            h_full = dram.tile([N_NODES, 128], f16)
            bounce = dram.tile([NPC, 128], f16)

            node_chunks = [(j * CW_N, min(CW_N, NPC - j * CW_N))
                           for j in range(-(-NPC // CW_N))]

            def epilogue(s):
                """h_state[s] -> row-major fp16 shard -> AllGather h_full."""
                for t in range(NW):
                    n0 = t * 128
                    tn = min(128, NPC - n0)
                    pst = ps_tr.tile([128, HID], f16, name="pst")
                    nc.tensor.transpose(pst[:tn, :],
                                        h_state[s][:, n0:n0 + tn], id96[:])
                    nc.scalar.copy(rm_buf[:tn, t, 0:HID], pst[:tn, :])
                nc.sync.dma_start(
                    out=bounce[0:(NW - 1) * 128, :].rearrange(
                        "(t p) d -> p t d", p=128),
                    in_=rm_buf[:, 0:NW - 1, :])
                tn = NPC - (NW - 1) * 128
                nc.sync.dma_start(out=bounce[(NW - 1) * 128:NPC, :],
                                  in_=rm_buf[:tn, NW - 1, :])
                nc.gpsimd.collective_compute(
                    "AllGather", mybir.AluOpType.bypass,
                    ins=[bounce.opt()], outs=[h_full.opt()],
                    replica_groups=[list(range(N_CORES))])

            # ---- input projection ----
            for n0, cw in node_chunks:
                xb = xpool.tile([IN_DIM, CW_N], f32r, name="xb")
                nc.sync.dma_start(out=xb[:, :cw],
                                  in_=xT_in[:, n0:n0 + cw].bitcast(f32r))
                ps = ps_big.tile([HID, CW_N], f32, name="psb")
                nc.tensor.matmul(ps[:, :cw], w_inT[:], xb[:, :cw],
                                 start=True, stop=True)
                nc.scalar.add(h_state[0][:, n0:n0 + cw], ps[:, :cw], b_in[:])
            epilogue(0)

            # ---- GIN layers ----
            h_lo = h_full[0:SPLIT, :]
            h_hi = h_full[SPLIT:N_NODES, :]
            w_chunks = [(c0, min(CHUNK_W, NW - c0))
                        for c0 in range(0, NW, CHUNK_W)]
            # gather unit: GT tiles (GT*128 idxs <= 1024 so the 64
            # descriptors per SDMA engine fit one packet), round-robin
            # over the 4 SWDGE queues (4 Q7 pairs generate + 4 rings
            # feed each engine concurrently)
            GT = 8
            qrr = [0]

            def emit_gathers(gbuf, src_view, idx_tile, base_tile, n_tiles):
                for s0 in range(0, n_tiles, GT):
                    sn = min(GT, n_tiles - s0)
                    nc.gpsimd.dma_gather(
                        gbuf[:, s0:s0 + sn, :], src_view,
                        idx_tile[:, (base_tile + s0) * 8:
                                 (base_tile + s0 + sn) * 8],
                        num_idxs=sn * 128, num_idxs_reg=sn * 128,
                        elem_size=128, single_packet=True,
                        queue_num=qrr[0] % 4)
                    qrr[0] += 1

            for l in range(N_LAYERS):
                for ci, (c0, cw) in enumerate(w_chunks):
                    g_lo = glo_pool.tile([128, CHUNK_W * T_lo, 128], f16,
                                         name="g_lo")
                    emit_gathers(g_lo, h_lo, idx_lo, c0 * T_lo, cw * T_lo)
                    g_hi = ghi_pool.tile([128, CHUNK_W * T_hi, 128], f16,
                                         name="g_hi")
                    emit_gathers(g_hi, h_hi, idx_hi, c0 * T_hi, cw * T_hi)
                    for wl in range(cw):
                        w_i = c0 + wl
                        n0 = w_i * 128
                        wn = min(128, NPC - n0)
                        oh = oh_pool.tile([128, T, WIN], f16, name="oh")
                        nc.vector.tensor_tensor(
                            oh[:],
                            iota[:].unsqueeze(1).broadcast_to([128, T, WIN]),
                            dstloc[:, w_i, :].unsqueeze(2)
                                .broadcast_to([128, T, WIN]),
                            mybir.AluOpType.is_equal)
                        ps = ps_agg.tile([HID, WIN], f32, name="psa")
                        for t in range(T):
                            if t < T_lo:
                                lhsT = g_lo[:, wl * T_lo + t, 0:HID]
                            else:
                                lhsT = g_hi[:, wl * T_hi + (t - T_lo), 0:HID]
                            nc.tensor.matmul(ps[:], lhsT, oh[:, t, :],
                                             start=(t == 0),
                                             stop=(t == T - 1))
                        nc.vector.tensor_tensor(
                            h_plus[:, n0:n0 + wn], ps[:, :wn],
                            h_state[l][:, n0:n0 + wn], mybir.AluOpType.add)
                for n0, cw in node_chunks:
                    ps = ps_big.tile([HID, CW_N], f32, name="psb")
                    nc.tensor.matmul(ps[:, :cw], w_lT[l][:],
                                     h_plus[:, n0:n0 + cw],
                                     start=True, stop=True)
                    nc.scalar.add(h_state[l + 1][:, n0:n0 + cw], ps[:, :cw],
                                  b_l[l][:])
                if l < N_LAYERS - 1:
                    epilogue(l + 1)

            # ---- output projection ----
            for n0, cw in node_chunks:
                ps = ps_big.tile([OUT_DIM, CW_N], f32, name="pso", tag="psb")
                for s in range(N_LAYERS + 1):
                    nc.tensor.matmul(ps[:, :cw], w_out4[s][:],
                                     h_state[s][:, n0:n0 + cw],
                                     start=(s == 0), stop=(s == N_LAYERS))
                ot = ot_pool.tile([OUT_DIM, CW_N], f32, name="ot")
                nc.scalar.add(ot[:, :cw], ps[:, :cw], b_out[:])
                for tt in range(-(-cw // 128)):
                    t0 = tt * 128
                    tn = min(128, cw - t0)
                    pst = ps_tr.tile([128, 128], f32, name="psto", tag="pst")
                    nc.tensor.transpose(pst[:tn, :], ot[:, t0:t0 + tn],
                                        id128[:])
                    orow = ot_pool.tile([128, 128], f32, name="orow")
                    nc.scalar.copy(orow[:tn, :], pst[:tn, :])
                    nc.sync.dma_start(
                        out=out_ext[n0 + t0:n0 + t0 + tn, :],
                        in_=orow[:tn, :])

    nc.compile()
    return nc


def _get_nc_and_inputs(inputs):
    from concourse import bass_utils  # noqa: F401  (path setup)

    x = np.asarray(inputs["x"], np.float32)
    edge_index = np.asarray(inputs["edge_index"], np.int32)
    W_in = np.asarray(inputs["W_in"], np.float32)
    b_in = np.asarray(inputs["b_in"], np.float32)
    W_layers = np.asarray(inputs["W_layers"], np.float32)
    b_layers = np.asarray(inputs["b_layers"], np.float32)
    W_out = np.asarray(inputs["W_out"], np.float32)
    b_out = np.asarray(inputs["b_out"], np.float32)

    idx_lo_w, idx_hi_w, dstloc, T_lo, T_hi, perm = _prep(edge_index)

    key = ("nc", T_lo, T_hi)
    if key not in _cache:
        _cache.clear()
        _cache[key] = _build(T_lo, T_hi)
    nc = _cache[key]

    inv = np.empty(N_NODES, np.int64)
    inv[perm] = np.arange(N_NODES)
    xT = np.ascontiguousarray(x.T[:, inv])
    w_inT = np.ascontiguousarray(W_in.T)
    w_lT = np.ascontiguousarray(W_layers.transpose(0, 2, 1))
    b_l = np.ascontiguousarray(b_layers[:, :, None])
    w_out4 = np.ascontiguousarray(
        np.stack([W_out[:, s * HID:(s + 1) * HID].T
                  for s in range(N_LAYERS + 1)])).astype(np.float16)
    iota = np.tile(np.arange(WIN, dtype=np.float16), (128, 1))
    id96 = np.eye(HID, dtype=np.float16)
    id128 = np.eye(128, dtype=np.float32)

    in_maps = []
    for c in range(N_CORES):
        in_maps.append({
            "xT": np.ascontiguousarray(xT[:, c * NPC:(c + 1) * NPC]),
            "w_inT": w_inT,
            "b_in": b_in.reshape(HID, 1),
            "w_lT": w_lT,
            "b_l": b_l,
            "w_out4": w_out4,
            "b_out": b_out.reshape(OUT_DIM, 1),
            "iota": iota,
            "id96": id96,
            "id128": id128,
            "idx_lo": idx_lo_w[c],
            "idx_hi": idx_hi_w[c],
            "dstloc": dstloc[c],
        })
    return nc, in_maps, perm


def run(inputs, trace=False):
    from concourse import bass_utils

    nc, in_maps, perm = _get_nc_and_inputs(inputs)
    res = bass_utils.run_bass_kernel_spmd(
        nc, in_maps, core_ids=list(range(N_CORES)), trace=trace)
    out = np.concatenate([res.results[c]["out"] for c in range(N_CORES)], 0)
    return out[perm], res


def kernel(**inputs):
    out, _ = run(inputs, trace=False)
    return out


# revision 13
# speedup vs baseline: 2.4818x; 1.0366x over previous
"""GIN-style 3-layer GNN encoder on 8 Trainium2 NeuronCores (Bass/Tile).

Reference computation (fp32):
    h = x @ W_in.T + b_in                                  [50000, 96]
    for l in 0..2:
        agg = segment_sum(h[src], dst, N)                  [50000, 96]
        h = (h + agg) @ W_layers[l].T + b_layers[l]
    out = concat([h0..h3], 1) @ W_out.T + b_out            [50000, 128]

Distribution: nodes are partitioned across the 8 cores (6250/core) via a
host-side balancing permutation; each edge is owned by the core that owns
its dst node.  Each layer the updated node features are AllGathered into
two replicated row-major fp16 tables h_fullA/h_fullB (first/second half
of every core's node range, 25000 x 256B rows each) — the split halves
the AllGather latency on the critical path (gathers from half A start
while AllGather B is still in flight) and keeps gather indices < 32768
(int16).

Per-core segment sum: a core's node range is split into 49 windows of 128
nodes.  Every window has a fixed number of 128-edge tiles (T_a tiles with
src in half A, T_b in half B; the balancing permutation equalizes
per-window per-class edge counts so the fixed tile counts are tight).
Edge features are fetched with gpsimd dma_gather (fp16 256B rows, 1024
idxs per instruction = one 64-descriptor packet per SDMA engine,
round-robin over the 4 SWDGE queues so all 4 Q7 pairs generate
descriptors concurrently).  For each window the one-hot
onehot[e, t, j] = (dst_local[e, t] == j) is built on DVE with one
broadcast is_equal, and the PE accumulates
    psum[96, 128] += gathered_tile[128e, 96].T @ onehot_tile[128e, 128]
which is aggT for the window.  h+agg (DVE), the layer matmul (fp32r),
bias (ACT), and a PE transpose to the row-major fp16 shard follow.
"""
import sys

sys.path.insert(0, "/opt/trn_rl_repo")

import numpy as np

N_NODES = 50000
N_EDGES = 800000
IN_DIM = 128
HID = 96
OUT_DIM = 128
N_LAYERS = 3
N_CORES = 8
NPC = N_NODES // N_CORES          # 6250 nodes per core
WIN = 128                         # window width (nodes)
NW = (NPC + WIN - 1) // WIN       # 49 windows per core (last = 106 nodes)
HALF = NPC // 2                   # 3125: per-core A/B split
CLS = N_CORES * HALF              # 25000: A-class size
AW = HALF // WIN                  # 24 full-A windows per core
REM_A = HALF - AW * WIN           # 53 A-slots in window 24
CHUNK_W = 5                       # windows per gather buffer
GT = 8                            # tiles per dma_gather (1024 idxs)
CW_N = 512                        # node-chunk for dense matmuls

_cache = {}


def _balance_nodes(src0, dst0):
    """Permute node ids so per-(core,window) A/B edge counts are even.

    A node's A/B class (which replicated gather table its row lives in) is
    frozen to its OLD id (< CLS -> A); the permutation only moves nodes
    within their class region, so per-node (deg_a, deg_b) are fixed and a
    greedy 2-D balance over the 392 (core, window) bins makes the uniform
    tile counts T_a/T_b tight.  Returns perm (old id -> new id).
    """
    deg_a = np.bincount(dst0[src0 < CLS], minlength=N_NODES).astype(np.int64)
    deg_b = np.bincount(dst0[src0 >= CLS], minlength=N_NODES).astype(np.int64)
    nbins = N_CORES * NW
    base = np.empty(nbins, np.int64)
    cap = np.empty(nbins, np.int64)
    for b in range(nbins):
        c, w = divmod(b, NW)
        base[b] = c * NPC + w * WIN
        cap[b] = min(WIN, NPC - w * WIN)
    woff = base % NPC
    q_a = np.clip(HALF - woff, 0, cap)   # A slots = first q_a of the window
    q_b = cap - q_a

    mu_a = max(1.0, deg_a.sum() / nbins)
    mu_b = max(1.0, deg_b.sum() / nbins)
    order = np.argsort(-(deg_a + deg_b), kind="stable")
    a_load = np.zeros(nbins)
    b_load = np.zeros(nbins)
    a_left = q_a.copy()
    b_left = q_b.copy()
    a_pos = np.zeros(nbins, np.int64)
    b_pos = q_a.copy()
    perm = np.empty(N_NODES, np.int64)
    for n in order:
        phi = np.maximum((a_load + deg_a[n]) / mu_a,
                         (b_load + deg_b[n]) / mu_b)
        if n < CLS:
            phi = np.where(a_left > 0, phi, np.inf)
            b_ = int(np.argmin(phi))
            perm[n] = base[b_] + a_pos[b_]
            a_pos[b_] += 1
            a_left[b_] -= 1
        else:
            phi = np.where(b_left > 0, phi, np.inf)
            b_ = int(np.argmin(phi))
            perm[n] = base[b_] + b_pos[b_]
            b_pos[b_] += 1
            b_left[b_] -= 1
        a_load[b_] += deg_a[n]
        b_load[b_] += deg_b[n]
    return perm


def _prep(edge_index):
    """Host-side edge bucketing -> per-core gather index / dst tables."""
    src0 = edge_index[0].astype(np.int64)
    dst0 = edge_index[1].astype(np.int64)
    perm = _balance_nodes(src0, dst0)
    src = perm[src0]
    dst = perm[dst0]
    core = dst // NPC
    din = dst % NPC
    w = din // WIN
    dstl = din % WIN
    s_in = src % NPC
    c_src = src // NPC
    is_b = (s_in >= HALF).astype(np.int64)
    pos = np.where(is_b == 0, c_src * HALF + s_in,
                   c_src * HALF + s_in - HALF)  # < 25000, int16-safe

    key = (core * NW + w) * 2 + is_b
    order = np.argsort(key, kind="stable")
    s_pos = pos[order]
    s_dstl = dstl[order]
    s_key = key[order]
    s_b = is_b[order]

    counts = np.bincount(key, minlength=N_CORES * NW * 2)
    T_a = max(1, int(-(-counts.reshape(-1, 2)[:, 0].max() // 128)))
    T_b = max(1, int(-(-counts.reshape(-1, 2)[:, 1].max() // 128)))
    T = T_a + T_b

    starts = np.zeros(N_CORES * NW * 2, np.int64)
    starts[1:] = np.cumsum(counts)[:-1]
    rank = np.arange(len(s_key)) - starts[s_key]

    c_arr = s_key // (2 * NW)
    w_arr = (s_key // 2) % NW

    idx_a = np.zeros((N_CORES, NW, T_a * 128), np.int16)
    idx_b = np.zeros((N_CORES, NW, T_b * 128), np.int16)
    dstl_arr = np.full((N_CORES, NW, T, 128), -1.0, np.float16)

    a_m = s_b == 0
    flat = (c_arr[a_m] * NW + w_arr[a_m]) * (T_a * 128) + rank[a_m]
    idx_a.reshape(-1)[flat] = s_pos[a_m].astype(np.int16)
    t_g = rank[a_m] // 128
    e_g = rank[a_m] % 128
    flat = ((c_arr[a_m] * NW + w_arr[a_m]) * T + t_g) * 128 + e_g
    dstl_arr.reshape(-1)[flat] = s_dstl[a_m].astype(np.float16)

    b_m = ~a_m
    flat = (c_arr[b_m] * NW + w_arr[b_m]) * (T_b * 128) + rank[b_m]
    idx_b.reshape(-1)[flat] = s_pos[b_m].astype(np.int16)
    t_g = rank[b_m] // 128 + T_a
    e_g = rank[b_m] % 128
    flat = ((c_arr[b_m] * NW + w_arr[b_m]) * T + t_g) * 128 + e_g
    dstl_arr.reshape(-1)[flat] = s_dstl[b_m].astype(np.float16)

    def wrap(vals):  # [NW*Tc*128] -> [128, NW*Tc*8] int16 wrapped+replicated
        v = vals.reshape(-1, 16).T
        return np.tile(v, (8, 1)).copy()

    idx_a_w = np.stack([wrap(idx_a[c].reshape(-1)) for c in range(N_CORES)])
    idx_b_w = np.stack([wrap(idx_b[c].reshape(-1)) for c in range(N_CORES)])
    dstloc = np.ascontiguousarray(dstl_arr.transpose(0, 3, 1, 2))  # [C,128,NW,T]
    return idx_a_w, idx_b_w, dstloc, T_a, T_b, perm


def _build(T_a, T_b):
    from concourse import bacc, tile, mybir, library_config

    dt = mybir.dt
    T = T_a + T_b
    nc = bacc.Bacc("TRN2", target_bir_lowering=False, debug=False,
                   num_devices=N_CORES, num_swdge_queues=4)

    # ---- I/O ----
    xT_in = nc.dram_tensor("xT", [IN_DIM, NPC], dt.float32, kind="ExternalInput")
    w_inT_in = nc.dram_tensor("w_inT", [IN_DIM, HID], dt.float32,
                              kind="ExternalInput")
    b_in_in = nc.dram_tensor("b_in", [HID, 1], dt.float32, kind="ExternalInput")
    w_lT_in = nc.dram_tensor("w_lT", [N_LAYERS, HID, HID], dt.float32,
                             kind="ExternalInput")
    b_l_in = nc.dram_tensor("b_l", [N_LAYERS, HID, 1], dt.float32,
                            kind="ExternalInput")
    w_out4_in = nc.dram_tensor("w_out4", [N_LAYERS + 1, HID, OUT_DIM],
                               dt.float16, kind="ExternalInput")
    b_out_in = nc.dram_tensor("b_out", [OUT_DIM, 1], dt.float32,
                              kind="ExternalInput")
    iota_in = nc.dram_tensor("iota", [128, WIN], dt.float16,
                             kind="ExternalInput")
    id96_in = nc.dram_tensor("id96", [HID, HID], dt.float16,
                             kind="ExternalInput")
    id128_in = nc.dram_tensor("id128", [128, 128], dt.float32,
                              kind="ExternalInput")
    idx_a_in = nc.dram_tensor("idx_a", [128, NW * T_a * 8], dt.int16,
                              kind="ExternalInput")
    idx_b_in = nc.dram_tensor("idx_b", [128, NW * T_b * 8], dt.int16,
                              kind="ExternalInput")
    dstloc_in = nc.dram_tensor("dstloc", [128, NW, T], dt.float16,
                               kind="ExternalInput")
    out_ext = nc.dram_tensor("out", [NPC, OUT_DIM], dt.float32,
                             kind="ExternalOutput")

    f32, f32r, f16 = dt.float32, dt.float32r, dt.float16

    with tile.TileContext(nc, num_cores=N_CORES) as tc:
        nc.gpsimd.load_library(library_config.mlp)
        with tc.tile_pool(name="persist", bufs=1) as pp, \
             tc.tile_pool(name="xpool", bufs=3) as xpool, \
             tc.tile_pool(name="ga", bufs=2) as ga_pool, \
             tc.tile_pool(name="gb", bufs=2) as gb_pool, \
             tc.tile_pool(name="oh", bufs=3) as oh_pool, \
             tc.tile_pool(name="otile", bufs=2) as ot_pool, \
             tc.tile_pool(name="ps_agg", bufs=3, space="PSUM") as ps_agg, \
             tc.tile_pool(name="ps_big", bufs=2, space="PSUM") as ps_big, \
             tc.tile_pool(name="ps_tr", bufs=2, space="PSUM") as ps_tr, \
             tc.tile_pool(name="dram", bufs=1, space="DRAM") as dram:

            def load(name, shape, dtype, src_ap):
                t = pp.tile(shape, dtype, name=name)
                nc.sync.dma_start(out=t[:], in_=src_ap)
                return t

            w_inT = load("w_inT", [IN_DIM, HID], f32r, w_inT_in[:].bitcast(f32r))
            b_in = load("b_in", [HID, 1], f32, b_in_in[:])
            w_lT = [load(f"w_lT{l}", [HID, HID], f32r, w_lT_in[l].bitcast(f32r))
                    for l in range(N_LAYERS)]
            b_l = [load(f"b_l{l}", [HID, 1], f32, b_l_in[l])
                   for l in range(N_LAYERS)]
            w_out4 = [load(f"w_out4_{s}", [HID, OUT_DIM], f16, w_out4_in[s])
                      for s in range(N_LAYERS + 1)]
            b_out = load("b_out", [OUT_DIM, 1], f32, b_out_in[:])
            iota = load("iota", [128, WIN], f16, iota_in[:])
            id96 = load("id96", [HID, HID], f16, id96_in[:])
            id128 = load("id128", [128, 128], f32, id128_in[:])
            idx_a = load("idx_a", [128, NW * T_a * 8], dt.int16, idx_a_in[:])
            idx_b = load("idx_b", [128, NW * T_b * 8], dt.int16, idx_b_in[:])
            dstloc = load("dstloc", [128, NW, T], f16, dstloc_in[:])

            h_state = [pp.tile([HID, NPC], f16, name=f"h{s}")
                       for s in range(N_LAYERS + 1)]
            h_plus = pp.tile([HID, NPC], f32r, name="h_plus")
            rm_buf = pp.tile([128, NW, 128], f16, name="rm_buf")

            h_fullA = dram.tile([CLS, 128], f16)
            h_fullB = dram.tile([N_NODES - CLS, 128], f16)
            bounceA = dram.tile([HALF, 128], f16)
            bounceB = dram.tile([NPC - HALF, 128], f16)

            node_chunks = [(j * CW_N, min(CW_N, NPC - j * CW_N))
                           for j in range(-(-NPC // CW_N))]

            def transpose_windows(s, w0, w1):
                for t in range(w0, w1):
                    n0 = t * 128
                    tn = min(128, NPC - n0)
                    pst = ps_tr.tile([128, HID], f16, name="pst")
                    nc.tensor.transpose(pst[:tn, :],
                                        h_state[s][:, n0:n0 + tn], id96[:])
                    nc.scalar.copy(rm_buf[:tn, t, 0:HID], pst[:tn, :])

            def epilogue(s):
                """h_state[s] -> row-major fp16 halves -> two AllGathers."""
                transpose_windows(s, 0, AW + 1)
                nc.sync.dma_start(
                    out=bounceA[0:AW * 128, :].rearrange(
                        "(t p) d -> p t d", p=128),
                    in_=rm_buf[:, 0:AW, :])
                nc.sync.dma_start(out=bounceA[AW * 128:HALF, :],
                                  in_=rm_buf[0:REM_A, AW, :])
                nc.gpsimd.collective_compute(
                    "AllGather", mybir.AluOpType.bypass,
                    ins=[bounceA.opt()], outs=[h_fullA.opt()],
                    replica_groups=[list(range(N_CORES))])
                transpose_windows(s, AW + 1, NW)
                nc.sync.dma_start(out=bounceB[0:WIN - REM_A, :],
                                  in_=rm_buf[REM_A:WIN, AW, :])
                nb_full = NW - AW - 2   # full windows AW+1 .. NW-2
                o0 = WIN - REM_A
                nc.sync.dma_start(
                    out=bounceB[o0:o0 + nb_full * 128, :].rearrange(
                        "(t p) d -> p t d", p=128),
                    in_=rm_buf[:, AW + 1:NW - 1, :])
                o1 = o0 + nb_full * 128
                last_n = NPC - (NW - 1) * WIN
                nc.sync.dma_start(out=bounceB[o1:o1 + last_n, :],
                                  in_=rm_buf[0:last_n, NW - 1, :])
                nc.gpsimd.collective_compute(
                    "AllGather", mybir.AluOpType.bypass,
                    ins=[bounceB.opt()], outs=[h_fullB.opt()],
                    replica_groups=[list(range(N_CORES))])

            # ---- input projection ----
            for n0, cw in node_chunks:
                xb = xpool.tile([IN_DIM, CW_N], f32r, name="xb")
                nc.sync.dma_start(out=xb[:, :cw],
                                  in_=xT_in[:, n0:n0 + cw].bitcast(f32r))
                ps = ps_big.tile([HID, CW_N], f32, name="psb")
                nc.tensor.matmul(ps[:, :cw], w_inT[:], xb[:, :cw],
                                 start=True, stop=True)
                nc.scalar.add(h_state[0][:, n0:n0 + cw], ps[:, :cw], b_in[:])
            epilogue(0)

            # ---- GIN layers ----
            qrr = [0]

            def emit_gathers(gbuf, src_view, idx_tile, base_tile, n_tiles):
                for s0 in range(0, n_tiles, GT):
                    sn = min(GT, n_tiles - s0)
                    nc.gpsimd.dma_gather(
                        gbuf[:, s0:s0 + sn, :], src_view,
                        idx_tile[:, (base_tile + s0) * 8:
                                 (base_tile + s0 + sn) * 8],
                        num_idxs=sn * 128, num_idxs_reg=sn * 128,
                        elem_size=128, single_packet=True,
                        queue_num=qrr[0] % 4)
                    qrr[0] += 1

            w_chunks = [(c0, min(CHUNK_W, NW - c0))
                        for c0 in range(0, NW, CHUNK_W)]
            for l in range(N_LAYERS):
                for c0, cw in w_chunks:
                    g_a = ga_pool.tile([128, CHUNK_W * T_a, 128], f16,
                                       name="g_a")
                    emit_gathers(g_a, h_fullA[:], idx_a, c0 * T_a, cw * T_a)
                    g_b = gb_pool.tile([128, CHUNK_W * T_b, 128], f16,
                                       name="g_b")
                    emit_gathers(g_b, h_fullB[:], idx_b, c0 * T_b, cw * T_b)
                    for wl in range(cw):
                        w_i = c0 + wl
                        n0 = w_i * 128
                        wn = min(128, NPC - n0)
                        oh = oh_pool.tile([128, T, WIN], f16, name="oh")
                        nc.vector.tensor_tensor(
                            oh[:],
                            iota[:].unsqueeze(1).broadcast_to([128, T, WIN]),
                            dstloc[:, w_i, :].unsqueeze(2)
                                .broadcast_to([128, T, WIN]),
                            mybir.AluOpType.is_equal)
                        ps = ps_agg.tile([HID, WIN], f32, name="psa")
                        for t in range(T):
                            if t < T_a:
                                lhsT = g_a[:, wl * T_a + t, 0:HID]
                            else:
                                lhsT = g_b[:, wl * T_b + (t - T_a), 0:HID]
                            nc.tensor.matmul(ps[:], lhsT, oh[:, t, :],
                                             start=(t == 0),
                                             stop=(t == T - 1))
                        nc.vector.tensor_tensor(
                            h_plus[:, n0:n0 + wn], ps[:, :wn],
                            h_state[l][:, n0:n0 + wn], mybir.AluOpType.add)
                for n0, cw in node_chunks:
                    ps = ps_big.tile([HID, CW_N], f32, name="psb")
                    nc.tensor.matmul(ps[:, :cw], w_lT[l][:],
                                     h_plus[:, n0:n0 + cw],
                                     start=True, stop=True)
                    nc.scalar.add(h_state[l + 1][:, n0:n0 + cw], ps[:, :cw],
                                  b_l[l][:])
                if l < N_LAYERS - 1:
                    epilogue(l + 1)

            # ---- output projection ----
            for n0, cw in node_chunks:
                ps = ps_big.tile([OUT_DIM, CW_N], f32, name="pso", tag="psb")
                for s in range(N_LAYERS + 1):
                    nc.tensor.matmul(ps[:, :cw], w_out4[s][:],
                                     h_state[s][:, n0:n0 + cw],
                                     start=(s == 0), stop=(s == N_LAYERS))
                ot = ot_pool.tile([OUT_DIM, CW_N], f32, name="ot")
                nc.scalar.add(ot[:, :cw], ps[:, :cw], b_out[:])
                for tt in range(-(-cw // 128)):
                    t0 = tt * 128
                    tn = min(128, cw - t0)
                    pst = ps_tr.tile([128, 128], f32, name="psto", tag="pst")
                    nc.tensor.transpose(pst[:tn, :], ot[:, t0:t0 + tn],
                                        id128[:])
                    orow = ot_pool.tile([128, 128], f32, name="orow")
                    nc.scalar.copy(orow[:tn, :], pst[:tn, :])
                    nc.sync.dma_start(
                        out=out_ext[n0 + t0:n0 + t0 + tn, :],
                        in_=orow[:tn, :])

    nc.compile()
    return nc


def _get_nc_and_inputs(inputs):
    from concourse import bass_utils  # noqa: F401  (path setup)

    x = np.asarray(inputs["x"], np.float32)
    edge_index = np.asarray(inputs["edge_index"], np.int32)
    W_in = np.asarray(inputs["W_in"], np.float32)
    b_in = np.asarray(inputs["b_in"], np.float32)
    W_layers = np.asarray(inputs["W_layers"], np.float32)
    b_layers = np.asarray(inputs["b_layers"], np.float32)
    W_out = np.asarray(inputs["W_out"], np.float32)
    b_out = np.asarray(inputs["b_out"], np.float32)

    idx_a_w, idx_b_w, dstloc, T_a, T_b, perm = _prep(edge_index)

    key = ("nc", T_a, T_b)
    if key not in _cache:
        _cache.clear()
        _cache[key] = _build(T_a, T_b)
    nc = _cache[key]

    inv = np.empty(N_NODES, np.int64)
    inv[perm] = np.arange(N_NODES)
    xT = np.ascontiguousarray(x.T[:, inv])
    w_inT = np.ascontiguousarray(W_in.T)
    w_lT = np.ascontiguousarray(W_layers.transpose(0, 2, 1))
    b_l = np.ascontiguousarray(b_layers[:, :, None])
    w_out4 = np.ascontiguousarray(
        np.stack([W_out[:, s * HID:(s + 1) * HID].T
                  for s in range(N_LAYERS + 1)])).astype(np.float16)
    iota = np.tile(np.arange(WIN, dtype=np.float16), (128, 1))
    id96 = np.eye(HID, dtype=np.float16)
    id128 = np.eye(128, dtype=np.float32)

    in_maps = []
    for c in range(N_CORES):
        in_maps.append({
            "xT": np.ascontiguousarray(xT[:, c * NPC:(c + 1) * NPC]),
            "w_inT": w_inT,
            "b_in": b_in.reshape(HID, 1),
            "w_lT": w_lT,
            "b_l": b_l,
            "w_out4": w_out4,
            "b_out": b_out.reshape(OUT_DIM, 1),
            "iota": iota,
            "id96": id96,
            "id128": id128,
            "idx_a": idx_a_w[c],
            "idx_b": idx_b_w[c],
            "dstloc": dstloc[c],
        })
    return nc, in_maps, perm


def run(inputs, trace=False):
    from concourse import bass_utils

    nc, in_maps, perm = _get_nc_and_inputs(inputs)
    res = bass_utils.run_bass_kernel_spmd(
        nc, in_maps, core_ids=list(range(N_CORES)), trace=trace)
    out = np.concatenate([res.results[c]["out"] for c in range(N_CORES)], 0)
    return out[perm], res


def kernel(**inputs):
    out, _ = run(inputs, trace=False)
    return out
